# revision 20
# baseline (speedup 1.0000x reference)
"""Multi-head self-attention on 8 Trainium2 NeuronCores.

Problem: x[2, 2048, 1024], 16 heads x 64 dim, fp32.
  qkv = x @ W_qkv + b_qkv ; attention per head ; out = attn @ W_out + b_out

Sharding: 8-way tensor parallel over heads — core c owns heads {2c, 2c+1}
for BOTH batches.  After each batch's attention, an 8-way AllToAll on
[8, 128, 256] blocks reshards head-split -> (batch, q)-split with NO
padding: block j of batch b's A2A carries this core's two heads for
global q columns [256j, 256j+256) of batch b.  Core c ends up owning
256 output rows of EACH batch and runs the output projection as two
independent passes (pass 0 = batch-0 rows, emitted before the second
collective completes so it overlaps the A2A; pass 1 = batch-1 rows).

Schedule highlights:
  - all matmul inputs bf16 except HALF the attention AV stage (k-chunks
    0..7), which runs fp8e4 with MatmulPerfMode.DoubleRow (2 contraction
    chunks per instruction): exp writes fp8 directly from the Scalar
    engine, v is cast to fp8 on the PSUM->SBUF transpose copies.  The
    other 8 chunks stay bf16 — the mix halves the fp8 noise (rel err
    1.5e-2 vs 2.1e-2 all-fp8) while the extra PE streaming mostly hides
    under the Scalar engine's exp, which paces the attention phase.
  - x arrives host-pre-transposed in an ns-major layout and is loaded
    with 4 chunked DMAs per batch (small weight DMAs first) so the first
    projection starts after ~1MB instead of 4MB.
  - k bias dropped exactly (softmax is invariant to per-query constants)
  - scores: four quadrant-packed K=64 matmuls per k-chunk co-execute
  - softmax denominator via ones-columns interleaved into v (free PE
    columns); normalize via reciprocal_approx_fast (5x faster than the
    iterative-divide reciprocal; must run at base partition 0) + multiply
  - projections/v-transposes of the other batch are interleaved into the
    attention iterations with per-iteration budgets chosen to keep the
    emitted PE work per scores-block under the Scalar engine's ~18us
  - pass 0 of the output projection reads a DVE-copied mirror of at0
    (precise engine semaphore) so Tile's conservative DMA-queue counting
    cannot chain it behind the second collective; the at0 gather itself
    is emitted late so its A2A#0-gated DMAs never sit ahead of b1's
    attention DMAs in a queue (head-of-line stall at the b0->b1 seam)
"""

import sys
import types

# ---------------------------------------------------------------------------
# antenv.axon_hooks shim: must exist BEFORE jax initializes so the axon boot
# registers the NTFF profiling hook into it (enables trace=True timing).
if "antenv.axon_hooks" not in sys.modules:
    _m = types.ModuleType("antenv.axon_hooks")
    _m._hook = None

    def _set_hook(h, _m=_m):
        _m._hook = h

    def _get_hook(_m=_m):
        return _m._hook

    _m.set_axon_ntff_profile_hook = _set_hook
    _m.get_axon_ntff_profile_hook = _get_hook
    sys.modules["antenv.axon_hooks"] = _m
    try:
        from trn_agent_boot.trn_boot import _ntff_profile_via_ctypes

        _h = _ntff_profile_via_ctypes("/opt/axon/libaxon_pjrt.so")
        if _h is not None:
            _m._hook = _h
    except Exception:
        pass

if "/opt/trn_rl_repo" not in sys.path:
    sys.path.insert(0, "/opt/trn_rl_repo")

import numpy as np

B, T, D, H, HD = 2, 2048, 1024, 16, 64
NC_ = 8
DC = D // 128          # 8 contraction chunks for the projections
TC = T // 128          # 16 k chunks
QS = 512               # attention q-slice width
QB = 256               # a2a block q width (per destination core per batch)
NQ = T // QS           # 4 q-slices per batch
SCALE = HD ** -0.5

FP8_AV = True          # AV stage in fp8e4 + DoubleRow

_CACHE = {}


def _build(trace_enabled=False):
    import concourse.bass as bass
    import concourse.mybir as mybir
    import concourse.tile as tile
    from concourse import bacc
    from concourse.masks import make_identity

    F32 = mybir.dt.float32
    BF16 = mybir.dt.bfloat16
    FP8 = mybir.dt.float8e4
    ETD = FP8 if FP8_AV else BF16
    EXPF = mybir.ActivationFunctionType.Exp
    DR = mybir.MatmulPerfMode.DoubleRow
    ADD = mybir.AluOpType.add

    nc = bacc.Bacc("TRN2", target_bir_lowering=False, debug=False, num_devices=NC_)

    xT_d = [nc.dram_tensor(f"xT{b}", [128, 4 * DC * QS], BF16, kind="ExternalInput")
            for b in range(B)]
    w_qk_d = nc.dram_tensor("w_qk", [128, DC * 256], BF16, kind="ExternalInput")
    b_qv_d = nc.dram_tensor("b_qv", [128, 2], F32, kind="ExternalInput")
    w_v_d = nc.dram_tensor("w_v", [128, DC * 128], BF16, kind="ExternalInput")
    w_out_d = nc.dram_tensor("w_out", [128, DC * D], BF16, kind="ExternalInput")
    bias_d = nc.dram_tensor("bias_bcast", [128, D], BF16, kind="ExternalInput")
    out_d = nc.dram_tensor("out", [512, D], BF16, kind="ExternalOutput")

    with tile.TileContext(nc) as tc:
        with (
            tc.tile_pool(name="const", bufs=1) as cpool,
            tc.tile_pool(name="qk", bufs=1) as qkpool,
            tc.tile_pool(name="vt", bufs=1) as vtpool,
            tc.tile_pool(name="v", bufs=1) as vpool,
            tc.tile_pool(name="exp", bufs=2) as epool,
            tc.tile_pool(name="small", bufs=3) as spool,
            tc.tile_pool(name="at", bufs=1) as atpool,
            tc.tile_pool(name="ps", bufs=4, space="PSUM") as ps,
            tc.tile_pool(name="ps2", bufs=2, space="PSUM") as ps2,
            tc.tile_pool(name="dram", bufs=1, space="DRAM") as dram,
        ):
            # ---- constants first (small, gate the first proj), then x ----
            xts = [cpool.tile([128, 4 * DC * QS], BF16, tag=f"xt{b}",
                              name=f"xt{b}") for b in range(B)]
            w_qk = cpool.tile([128, DC * 256], BF16, tag="wqk")
            nc.sync.dma_start(w_qk[:], w_qk_d[:, :])
            w_v = cpool.tile([128, DC * 128], BF16, tag="wv")
            nc.sync.dma_start(w_v[:], w_v_d[:, :])
            # per-partition bias columns: col 0 = q bias, col 1 = v bias
            b_qv = cpool.tile([128, 2], F32, tag="bqv")
            nc.sync.dma_start(b_qv[:], b_qv_d[:, :])
            for ns in range(NQ):
                nc.sync.dma_start(xts[0][:, 4096 * ns:4096 * (ns + 1)],
                                  xT_d[0][:, 4096 * ns:4096 * (ns + 1)])
            zt = cpool.tile([128, QS], BF16, tag="zt")
            nc.vector.memset(zt[:], 0.0)
            zcol = cpool.tile([128, 1], F32, tag="zcol")
            nc.vector.memset(zcol[:], 0.0)
            ident = cpool.tile([128, 128], BF16, tag="ident")
            make_identity(nc, ident[:])
            for ns in range(NQ):
                nc.sync.dma_start(xts[1][:, 4096 * ns:4096 * (ns + 1)],
                                  xT_d[1][:, 4096 * ns:4096 * (ns + 1)])
            w_out = cpool.tile([128, DC * D], BF16, tag="wout")
            nc.sync.dma_start(w_out[:], w_out_d[:, :])
            bias_b = cpool.tile([128, D], BF16, tag="biasb")
            nc.sync.dma_start(bias_b[:], bias_d[:, :])

            a2a_in = [dram.tile([NC_, 128, QB], BF16, name=f"a2a_in{b}")
                      for b in range(B)]
            a2a_out = [dram.tile([NC_, 128, QB], BF16, name=f"a2a_out{b}")
                       for b in range(B)]

            qks = [qkpool.tile([128, 2 * T], BF16, tag=f"qk{b}", name=f"qk{b}")
                   for b in range(B)]
            vts = [vtpool.tile([128, T], BF16, tag=f"vt{b}", name=f"vt{b}")
                   for b in range(B)]
            # Mixed-precision AV: k-chunks 0..7 (pairs 0..3) run fp8e4
            # DoubleRow, chunks 8..15 run bf16 — halves the fp8 noise while
            # most of the extra PE streaming hides under the Scalar engine.
            # v8 layout per pair p8 (k chunks 2p8, 2p8+1 = A, B):
            #   [A_h0 64 | ones 64 | B_h0 | ones | A_h1 | ones | B_h1 | ones]
            # v16 layout per chunk j (kc = 8+j):
            #   [ch_h0 64 | ones 64 | ch_h1 64 | ones 64]
            v8s = [vpool.tile([128, 4 * 512], FP8, tag=f"v8{b}", name=f"v8{b}")
                   for b in range(B)]
            v16s = [vpool.tile([128, 8 * 256], BF16, tag=f"v16{b}",
                               name=f"v16{b}") for b in range(B)]
            at_t = [atpool.tile([128, NC_ * QB], BF16, tag=f"at{b}",
                                name=f"at{b}") for b in range(B)]
            # DVE-copied mirror of at0: pass-0's weight loads then wait on a
            # precise engine semaphore instead of conservative DMA-queue
            # counts (which chain behind the A2A#1-gated at1 gathers).
            at0c = atpool.tile([128, NC_ * QB], BF16, tag="at0c", name="at0c")

            def proj_block(bi, kind, ns):
                """One [128, 512] projection block: kind 0=q, 1=k, 2=v.
                q/v biases ride the PSUM->SBUF copy on the DVE; k needs no
                bias (softmax drops per-query constants)."""
                xt, qk, vt = xts[bi], qks[bi], vts[bi]
                p = ps.tile([128, QS], F32, tag="ps", name="pproj")
                for dc in range(DC):
                    if kind < 2:
                        lhsT = w_qk[:, 256 * dc + 128 * kind:
                                    256 * dc + 128 * kind + 128]
                    else:
                        lhsT = w_v[:, 128 * dc:128 * (dc + 1)]
                    nc.tensor.matmul(
                        p[:], lhsT=lhsT,
                        rhs=xt[:, 4096 * ns + 512 * dc:4096 * ns + 512 * (dc + 1)],
                        start=(dc == 0), stop=(dc == DC - 1))
                dst = vt if kind == 2 else qk
                off = QS * ns if kind == 2 else T * kind + QS * ns
                if kind == 1:
                    nc.vector.tensor_copy(dst[:, off:off + QS], p[:])
                else:
                    nc.vector.scalar_tensor_tensor(
                        dst[:, off:off + QS], p[:],
                        b_qv[:, kind // 2:kind // 2 + 1], zt[:],
                        ADD, ADD)

            def v_transpose_chunk(bi, kc):
                vt = vts[bi]
                if kc == 0:
                    nc.vector.memset(v8s[bi][:], 1.0)
                    nc.vector.memset(v16s[bi][:], 1.0)
                pt = ps.tile([128, 128], BF16, tag="ps", name="pt")
                nc.tensor.transpose(pt[:], vt[:, 128 * kc:128 * (kc + 1)],
                                    ident[:])
                if kc < 8:
                    v = v8s[bi]
                    p8, a = kc // 2, kc % 2
                    for h in range(2):
                        col = 512 * p8 + 256 * h + 128 * a
                        nc.vector.tensor_copy(v[:, col:col + 64],
                                              pt[:, 64 * h:64 * h + 64])
                else:
                    v = v16s[bi]
                    j = kc - 8
                    for h in range(2):
                        col = 256 * j + 128 * h
                        nc.vector.tensor_copy(v[:, col:col + 64],
                                              pt[:, 64 * h:64 * h + 64])

            # etA (fp8) layout per pair p8, 2048 cols: [A_h0 512|B_h0|A_h1|B_h1]
            # etB (bf16) layout per chunk j, 1024 cols: [h0 512 | h1 512]
            def scores_block(bi, qs, etA, etB):
                qk = qks[bi]
                for kc in range(TC):
                    psc = ps2.tile([128, 2 * QS], F32, tag="ps2", name="psc")
                    kb = T + 128 * kc
                    for h in range(2):
                        po = 64 * h
                        nc.tensor.matmul(
                            psc[0:64, QS * h:QS * (h + 1)],
                            lhsT=qk[po:po + 64, kb:kb + 64],
                            rhs=qk[po:po + 64, QS * qs:QS * (qs + 1)],
                            start=True, stop=True,
                            tile_position=(po, 0))
                        nc.tensor.matmul(
                            psc[64:128, QS * h:QS * (h + 1)],
                            lhsT=qk[po:po + 64, kb + 64:kb + 128],
                            rhs=qk[po:po + 64, QS * qs:QS * (qs + 1)],
                            start=True, stop=True,
                            tile_position=(po, 64))
                    if kc < 8:
                        p8, a = kc // 2, kc % 2
                        # strided write: chunk a of both heads' segments
                        dst = etA[:, 2048 * p8:2048 * (p8 + 1)].rearrange(
                            "p (h two n) -> p h two n", h=2, two=2)[:, :, a, :]
                        nc.scalar.activation(
                            dst, psc[:].rearrange("p (h n) -> p h n", h=2),
                            EXPF, scale=SCALE)
                    else:
                        j = kc - 8
                        nc.scalar.activation(
                            etB[:, 1024 * j:1024 * (j + 1)], psc[:],
                            EXPF, scale=SCALE)

            def emit_tail(prev):
                pbi, pqs, petA, petB = prev
                pv8, pv16 = v8s[pbi], v16s[pbi]
                for h in range(2):
                    pav = ps.tile([128, QS], F32, tag="ps", name="pav")
                    for p8 in range(4):
                        lhsT = pv8[:, 512 * p8 + 256 * h:
                                   512 * p8 + 256 * h + 256].rearrange(
                            "p (two m) -> p two m", two=2)
                        rhs = petA[:, 2048 * p8 + 1024 * h:
                                   2048 * p8 + 1024 * (h + 1)].rearrange(
                            "p (two n) -> p two n", two=2)
                        nc.tensor.matmul(
                            pav[:], lhsT=lhsT, rhs=rhs,
                            start=(p8 == 0), stop=False,
                            perf_mode=DR)
                    for j in range(8):
                        nc.tensor.matmul(
                            pav[:],
                            lhsT=pv16[:, 256 * j + 128 * h:
                                      256 * j + 128 * (h + 1)],
                            rhs=petB[:, 1024 * j + 512 * h:
                                     1024 * j + 512 * (h + 1)],
                            start=False, stop=(j == 7))
                    rt = spool.tile([128, QS], F32, tag="rt", name="rt")
                    # NOTE: reciprocal_approx_fast requires base partition 0
                    # (custom-DVE ucode) — run full-range; rows 0:64 are
                    # garbage reciprocals of the numerators and never read.
                    nc.vector.reciprocal_approx_fast(
                        out=rt[:], in_=pav[:])
                    ot = spool.tile([128, QS], BF16, tag="ot", name="ot")
                    nc.vector.tensor_mul(ot[0:64, :], pav[0:64, :],
                                         rt[64:128, :])
                    for half in range(2):
                        nc.sync.dma_start(
                            a2a_in[pbi][2 * pqs + half, 64 * h:64 * h + 64, :],
                            ot[0:64, QB * half:QB * (half + 1)])

            def out_pass(half):
                at = at0c if half == 0 else at_t[half]
                for qc in range(2):
                    for ns in range(2):
                        p = ps.tile([128, QS], F32, tag="ps", name="pout")
                        for cc in range(NC_):
                            nc.tensor.matmul(
                                p[:],
                                lhsT=at[:, QB * cc + 128 * qc:
                                        QB * cc + 128 * (qc + 1)],
                                rhs=w_out[:, D * cc + QS * ns:
                                          D * cc + QS * (ns + 1)],
                                start=(cc == 0), stop=(cc == NC_ - 1))
                        os_ = spool.tile([128, QS], BF16, tag="os", name="os")
                        nc.vector.scalar_tensor_tensor(
                            os_[:], p[:], zcol[:],
                            bias_b[:, QS * ns:QS * (ns + 1)],
                            ADD, ADD)
                        nc.sync.dma_start(
                            out_d[QB * half + 128 * qc:
                                  QB * half + 128 * (qc + 1),
                                  QS * ns:QS * (ns + 1)],
                            os_[:])

            # ---- emission schedule --------------------------------------
            def new_et():
                etA = epool.tile([128, 4 * 2048], FP8, tag="etA", name="etA")
                etB = epool.tile([128, 8 * 1024], BF16, tag="etB", name="etB")
                return etA, etB

            proj_block(0, 1, 0)             # b0 k0
            proj_block(0, 0, 0)             # b0 q0 (needs only x chunk 0)
            for ns in range(1, NQ):
                proj_block(0, 1, ns)        # b0 k1..k3

            # Interleave slots per attention iteration, balanced so the PE
            # work between consecutive scores_blocks stays under the Scalar
            # engine's ~18us per-iteration exp time (pre-items are emitted
            # before the deferred tail, post-items after).
            kindmap = {"q": 0, "k": 1, "v": 2}

            def do_items(items):
                for tag_, arg in items:
                    if tag_.startswith("T"):
                        bi = int(tag_[1])
                        for kc in range(*arg):
                            v_transpose_chunk(bi, kc)
                    else:
                        proj_block(int(tag_[1]), kindmap[tag_[0]], arg)

            b0_pre = [
                [],
                [("v0", 2), ("v0", 3), ("T0", (8, 16))],
                [],
                [],
            ]
            b0_post = [
                [("q0", 1), ("v0", 0), ("v0", 1), ("T0", (0, 8))],
                [("q0", 2)],
                [("q0", 3), ("k1", 0), ("k1", 1)],
                [("k1", 2), ("k1", 3), ("q1", 0)],
            ]
            prev = None
            for qs in range(NQ):
                etA, etB = new_et()
                scores_block(0, qs, etA, etB)
                do_items(b0_pre[qs])
                if prev is not None:
                    emit_tail(prev)
                prev = (0, qs, etA, etB)
                do_items(b0_post[qs])

            b1_pre = [
                [],
                [("v1", 2), ("v1", 3), ("T1", (8, 16))],
                [],
                [],
            ]
            b1_post = [
                [("v1", 0), ("v1", 1), ("T1", (0, 8)), ("q1", 1)],
                [("q1", 2)],
                [("q1", 3)],
                [],
            ]
            for qs in range(NQ):
                etA, etB = new_et()
                scores_block(1, qs, etA, etB)
                do_items(b1_pre[qs])
                emit_tail(prev)             # (b0,3) at qs==0
                if qs == 0:
                    nc.gpsimd.collective_compute(
                        "AllToAll", mybir.AluOpType.bypass,
                        replica_groups=[list(range(NC_))],
                        ins=[a2a_in[0].opt()], outs=[a2a_out[0].opt()])
                if qs == 3:
                    # Gather at0 late: these DMAs are gated on A2A#0, and
                    # emitting them early lets the scheduler queue them ahead
                    # of b1's attention DMAs -> head-of-line stall at the
                    # b0->b1 seam.  Here only tail(b1,3) follows them, and by
                    # then A2A#0 has long completed.
                    for cc in range(NC_):
                        nc.sync.dma_start(at_t[0][:, QB * cc:QB * (cc + 1)],
                                          a2a_out[0][cc, :, :])
                    nc.vector.tensor_copy(at0c[:], at_t[0][:])
                prev = (1, qs, etA, etB)
                do_items(b1_post[qs])

            emit_tail(prev)                 # (b1,3)
            nc.gpsimd.collective_compute(
                "AllToAll", mybir.AluOpType.bypass,
                replica_groups=[list(range(NC_))],
                ins=[a2a_in[1].opt()], outs=[a2a_out[1].opt()])
            out_pass(0)                     # batch-0 rows; overlaps A2A#1
            # Deprioritize the at1 gather so the Tile scheduler cannot place
            # these (A2A#1-gated) DMAs ahead of pass-0's weight loads on
            # shared queues — that head-of-line ordering serialized pass 0
            # behind the second collective.
            with tc.high_priority(offset=-1000000):
                for cc in range(NC_):
                    nc.sync.dma_start(at_t[1][:, QB * cc:QB * (cc + 1)],
                                      a2a_out[1][cc, :, :])
            out_pass(1)                     # batch-1 rows

    nc.compile()
    return nc


def _chunked(a):
    """[DC*128, C] -> [128, DC*C] with chunk dc = rows 128dc:128(dc+1)."""
    r, c = a.shape
    return np.ascontiguousarray(
        a.reshape(DC, 128, c).transpose(1, 0, 2).reshape(128, DC * c))


def _shard_inputs(x, W_qkv, b_qkv, W_out, b_out):
    import ml_dtypes

    bf16 = ml_dtypes.bfloat16
    # ns-major x: xt[p, (ns*8+dc)*512 + t] = x[b][512ns+t, 128dc+p]
    xT = []
    for b in range(B):
        a = np.asarray(x[b], np.float32).astype(bf16)
        xT.append(np.ascontiguousarray(
            a.reshape(4, 512, 8, 128).transpose(3, 0, 2, 1).reshape(128, 16384)))
    W_out_bf = _chunked(W_out.astype(bf16))
    bias_bcast = np.ascontiguousarray(
        np.broadcast_to(b_out[None, :].astype(bf16), (128, D)))
    in_maps = []
    for c in range(NC_):
        lo = 64 * (2 * c)          # first channel of this core's 2 heads
        w_qk_c = _chunked(
            np.concatenate([W_qkv[:, lo:lo + 128],
                            W_qkv[:, D + lo:D + lo + 128]],
                           axis=1).astype(bf16))
        b_qv_c = np.ascontiguousarray(
            np.stack([b_qkv[lo:lo + 128],
                      b_qkv[2 * D + lo:2 * D + lo + 128]],
                     axis=1).astype(np.float32))
        w_v_c = _chunked(W_qkv[:, 2 * D + lo:2 * D + lo + 128].astype(bf16))
        in_maps.append({
            "xT0": xT[0], "xT1": xT[1],
            "w_qk": w_qk_c,
            "b_qv": b_qv_c,
            "w_v": w_v_c,
            "w_out": W_out_bf, "bias_bcast": bias_bcast,
        })
    return in_maps


def _run(inputs, trace=False, trace_kwargs=None):
    from concourse.bass_utils import run_bass_kernel_spmd

    if "nc" not in _CACHE:
        _CACHE["nc"] = _build()
    nc = _CACHE["nc"]
    in_maps = _shard_inputs(inputs["x"], inputs["W_qkv"], inputs["b_qkv"],
                            inputs["W_out"], inputs["b_out"])
    res = run_bass_kernel_spmd(nc, in_maps, core_ids=list(range(NC_)),
                               trace=trace, **(trace_kwargs or {}))
    out = np.empty((B, T, D), dtype=np.float32)
    for c in range(NC_):
        r = np.asarray(res.results[c]["out"]).astype(np.float32)
        out[0, QB * c:QB * (c + 1), :] = r[0:QB, :]
        out[1, QB * c:QB * (c + 1), :] = r[QB:2 * QB, :]
    return out, res


def kernel(x, mask, W_qkv, b_qkv, W_out, b_out):
    out, _ = _run({"x": np.asarray(x, dtype=np.float32),
                   "W_qkv": np.asarray(W_qkv, dtype=np.float32),
                   "b_qkv": np.asarray(b_qkv, dtype=np.float32),
                   "W_out": np.asarray(W_out, dtype=np.float32),
                   "b_out": np.asarray(b_out, dtype=np.float32)})
    return out


# revision 22
# speedup vs baseline: 1.0342x; 1.0342x over previous
"""Multi-head self-attention on 8 Trainium2 NeuronCores.

Problem: x[2, 2048, 1024], 16 heads x 64 dim, fp32.
  qkv = x @ W_qkv + b_qkv ; attention per head ; out = attn @ W_out + b_out

Sharding: 8-way tensor parallel over heads — core c owns heads {2c, 2c+1}
for BOTH batches.  After each batch's attention, an 8-way AllToAll on
[8, 128, 256] blocks reshards head-split -> (batch, q)-split with NO
padding: block j of batch b's A2A carries this core's two heads for
global q columns [256j, 256j+256) of batch b.  Core c ends up owning
256 output rows of EACH batch and runs the output projection as two
independent passes (pass 0 = batch-0 rows, emitted before the second
collective completes so it overlaps the A2A; pass 1 = batch-1 rows).

Schedule highlights:
  - all matmul inputs bf16 except HALF the attention AV stage (k-chunks
    0..7), which runs fp8e4 with MatmulPerfMode.DoubleRow (2 contraction
    chunks per instruction): exp writes fp8 directly from the Scalar
    engine, v is cast to fp8 on the PSUM->SBUF transpose copies.  The
    other 8 chunks stay bf16 — the mix halves the fp8 noise (rel err
    1.5e-2 vs 2.1e-2 all-fp8) while the extra PE streaming mostly hides
    under the Scalar engine's exp, which paces the attention phase.
  - x arrives host-pre-transposed in an ns-major layout and is loaded
    with 4 chunked DMAs per batch (small weight DMAs first) so the first
    projection starts after ~1MB instead of 4MB.
  - k bias dropped exactly (softmax is invariant to per-query constants)
  - scores: four quadrant-packed K=64 matmuls per k-chunk co-execute
  - softmax denominator via ones-columns interleaved into v (free PE
    columns); normalize via reciprocal_approx_fast (5x faster than the
    iterative-divide reciprocal; must run at base partition 0) + multiply
  - projections/v-transposes of the other batch are interleaved into the
    attention iterations with per-iteration budgets chosen to keep the
    emitted PE work per scores-block under the Scalar engine's ~18us
  - pass 0 of the output projection reads a DVE-copied mirror of at0
    (precise engine semaphore) so Tile's conservative DMA-queue counting
    cannot chain it behind the second collective; the at0 gather itself
    is emitted late so its A2A#0-gated DMAs never sit ahead of b1's
    attention DMAs in a queue (head-of-line stall at the b0->b1 seam)
"""

import sys
import types

# ---------------------------------------------------------------------------
# antenv.axon_hooks shim: must exist BEFORE jax initializes so the axon boot
# registers the NTFF profiling hook into it (enables trace=True timing).
if "antenv.axon_hooks" not in sys.modules:
    _m = types.ModuleType("antenv.axon_hooks")
    _m._hook = None

    def _set_hook(h, _m=_m):
        _m._hook = h

    def _get_hook(_m=_m):
        return _m._hook

    _m.set_axon_ntff_profile_hook = _set_hook
    _m.get_axon_ntff_profile_hook = _get_hook
    sys.modules["antenv.axon_hooks"] = _m
    try:
        from trn_agent_boot.trn_boot import _ntff_profile_via_ctypes

        _h = _ntff_profile_via_ctypes("/opt/axon/libaxon_pjrt.so")
        if _h is not None:
            _m._hook = _h
    except Exception:
        pass

if "/opt/trn_rl_repo" not in sys.path:
    sys.path.insert(0, "/opt/trn_rl_repo")

import numpy as np

B, T, D, H, HD = 2, 2048, 1024, 16, 64
NC_ = 8
DC = D // 128          # 8 contraction chunks for the projections
TC = T // 128          # 16 k chunks
QS = 512               # attention q-slice width
QB = 256               # a2a block q width (per destination core per batch)
NQ = T // QS           # 4 q-slices per batch
SCALE = HD ** -0.5

FP8_AV = True          # AV stage in fp8e4 + DoubleRow

_CACHE = {}


def _build(trace_enabled=False):
    import concourse.bass as bass
    import concourse.mybir as mybir
    import concourse.tile as tile
    from concourse import bacc
    from concourse.masks import make_identity

    F32 = mybir.dt.float32
    BF16 = mybir.dt.bfloat16
    FP8 = mybir.dt.float8e4
    ETD = FP8 if FP8_AV else BF16
    EXPF = mybir.ActivationFunctionType.Exp
    DR = mybir.MatmulPerfMode.DoubleRow
    ADD = mybir.AluOpType.add

    nc = bacc.Bacc("TRN2", target_bir_lowering=False, debug=False, num_devices=NC_)

    xT_d = [nc.dram_tensor(f"xT{b}", [128, 4 * DC * QS], BF16, kind="ExternalInput")
            for b in range(B)]
    w_qk_d = nc.dram_tensor("w_qk", [128, DC * 256], BF16, kind="ExternalInput")
    b_qv_d = nc.dram_tensor("b_qv", [128, 2], F32, kind="ExternalInput")
    w_v_d = nc.dram_tensor("w_v", [128, DC * 128], BF16, kind="ExternalInput")
    w_out_d = nc.dram_tensor("w_out", [128, DC * D], BF16, kind="ExternalInput")
    bias_d = nc.dram_tensor("bias_bcast", [128, D], BF16, kind="ExternalInput")
    out_d = nc.dram_tensor("out", [512, D], BF16, kind="ExternalOutput")

    with tile.TileContext(nc) as tc:
        with (
            tc.tile_pool(name="const", bufs=1) as cpool,
            tc.tile_pool(name="qk", bufs=1) as qkpool,
            tc.tile_pool(name="vt", bufs=1) as vtpool,
            tc.tile_pool(name="v", bufs=1) as vpool,
            tc.tile_pool(name="exp", bufs=2) as epool,
            tc.tile_pool(name="small", bufs=3) as spool,
            tc.tile_pool(name="at", bufs=1) as atpool,
            tc.tile_pool(name="ps", bufs=4, space="PSUM") as ps,
            tc.tile_pool(name="ps2", bufs=2, space="PSUM") as ps2,
            tc.tile_pool(name="dram", bufs=1, space="DRAM") as dram,
        ):
            # ---- constants first (small, gate the first proj), then x ----
            xts = [cpool.tile([128, 4 * DC * QS], BF16, tag=f"xt{b}",
                              name=f"xt{b}") for b in range(B)]
            w_qk = cpool.tile([128, DC * 256], BF16, tag="wqk")
            nc.sync.dma_start(w_qk[:], w_qk_d[:, :])
            w_v = cpool.tile([128, DC * 128], BF16, tag="wv")
            nc.sync.dma_start(w_v[:], w_v_d[:, :])
            # per-partition bias columns: col 0 = q bias, col 1 = v bias
            b_qv = cpool.tile([128, 2], F32, tag="bqv")
            nc.sync.dma_start(b_qv[:], b_qv_d[:, :])
            for ns in range(NQ):
                nc.sync.dma_start(xts[0][:, 4096 * ns:4096 * (ns + 1)],
                                  xT_d[0][:, 4096 * ns:4096 * (ns + 1)])
            zt = cpool.tile([128, QS], BF16, tag="zt")
            nc.vector.memset(zt[:], 0.0)
            zcol = cpool.tile([128, 1], F32, tag="zcol")
            nc.vector.memset(zcol[:], 0.0)
            ident = cpool.tile([128, 128], BF16, tag="ident")
            make_identity(nc, ident[:])
            for ns in range(NQ):
                nc.sync.dma_start(xts[1][:, 4096 * ns:4096 * (ns + 1)],
                                  xT_d[1][:, 4096 * ns:4096 * (ns + 1)])
            w_out = cpool.tile([128, DC * D], BF16, tag="wout")
            nc.sync.dma_start(w_out[:], w_out_d[:, :])
            bias_b = cpool.tile([128, D], BF16, tag="biasb")
            nc.sync.dma_start(bias_b[:], bias_d[:, :])

            a2a_in = [dram.tile([NC_, 128, QB], BF16, name=f"a2a_in{b}")
                      for b in range(B)]
            a2a_out = [dram.tile([NC_, 128, QB], BF16, name=f"a2a_out{b}")
                       for b in range(B)]

            qks = [qkpool.tile([128, 2 * T], BF16, tag=f"qk{b}", name=f"qk{b}")
                   for b in range(B)]
            vts = [vtpool.tile([128, T], BF16, tag=f"vt{b}", name=f"vt{b}")
                   for b in range(B)]
            # Mixed-precision AV: k-chunks 0..7 (pairs 0..3) run fp8e4
            # DoubleRow, chunks 8..15 run bf16 — halves the fp8 noise while
            # most of the extra PE streaming hides under the Scalar engine.
            # v8 layout per pair p8 (k chunks 2p8, 2p8+1 = A, B):
            #   [A_h0 64 | ones 64 | B_h0 | ones | A_h1 | ones | B_h1 | ones]
            # v16 layout per chunk j (kc = 8+j):
            #   [ch_h0 64 | ones 64 | ch_h1 64 | ones 64]
            v8s = [vpool.tile([128, 4 * 512], FP8, tag=f"v8{b}", name=f"v8{b}")
                   for b in range(B)]
            v16s = [vpool.tile([128, 8 * 256], BF16, tag=f"v16{b}",
                               name=f"v16{b}") for b in range(B)]
            at_t = [atpool.tile([128, NC_ * QB], BF16, tag=f"at{b}",
                                name=f"at{b}") for b in range(B)]
            # DVE-copied mirror of at0: pass-0's weight loads then wait on a
            # precise engine semaphore instead of conservative DMA-queue
            # counts (which chain behind the A2A#1-gated at1 gathers).
            at0c = atpool.tile([128, NC_ * QB], BF16, tag="at0c", name="at0c")

            def proj_block(bi, kind, ns):
                """One [128, 512] projection block: kind 0=q, 1=k, 2=v.
                q/v biases ride the PSUM->SBUF copy on the DVE; k needs no
                bias (softmax drops per-query constants)."""
                xt, qk, vt = xts[bi], qks[bi], vts[bi]
                p = ps.tile([128, QS], F32, tag="ps", name="pproj")
                for dc in range(DC):
                    if kind < 2:
                        lhsT = w_qk[:, 256 * dc + 128 * kind:
                                    256 * dc + 128 * kind + 128]
                    else:
                        lhsT = w_v[:, 128 * dc:128 * (dc + 1)]
                    nc.tensor.matmul(
                        p[:], lhsT=lhsT,
                        rhs=xt[:, 4096 * ns + 512 * dc:4096 * ns + 512 * (dc + 1)],
                        start=(dc == 0), stop=(dc == DC - 1))
                dst = vt if kind == 2 else qk
                off = QS * ns if kind == 2 else T * kind + QS * ns
                if kind == 1:
                    nc.vector.tensor_copy(dst[:, off:off + QS], p[:])
                else:
                    nc.vector.scalar_tensor_tensor(
                        dst[:, off:off + QS], p[:],
                        b_qv[:, kind // 2:kind // 2 + 1], zt[:],
                        ADD, ADD)

            def v_transpose_chunk(bi, kc):
                vt = vts[bi]
                if kc == 0:
                    nc.vector.memset(v8s[bi][:], 1.0)
                    nc.vector.memset(v16s[bi][:], 1.0)
                pt = ps.tile([128, 128], BF16, tag="ps", name="pt")
                nc.tensor.transpose(pt[:], vt[:, 128 * kc:128 * (kc + 1)],
                                    ident[:])
                if kc < 8:
                    v = v8s[bi]
                    p8, a = kc // 2, kc % 2
                    for h in range(2):
                        col = 512 * p8 + 256 * h + 128 * a
                        nc.vector.tensor_copy(v[:, col:col + 64],
                                              pt[:, 64 * h:64 * h + 64])
                else:
                    v = v16s[bi]
                    j = kc - 8
                    for h in range(2):
                        col = 256 * j + 128 * h
                        nc.vector.tensor_copy(v[:, col:col + 64],
                                              pt[:, 64 * h:64 * h + 64])

            # etA (fp8) layout per pair p8, 2048 cols: [A_h0 512|B_h0|A_h1|B_h1]
            # etB (bf16) layout per chunk j, 1024 cols: [h0 512 | h1 512]
            def scores_block(bi, qs, etA, etB):
                qk = qks[bi]
                for kc in range(TC):
                    psc = ps2.tile([128, 2 * QS], F32, tag="ps2", name="psc")
                    kb = T + 128 * kc
                    for h in range(2):
                        po = 64 * h
                        nc.tensor.matmul(
                            psc[0:64, QS * h:QS * (h + 1)],
                            lhsT=qk[po:po + 64, kb:kb + 64],
                            rhs=qk[po:po + 64, QS * qs:QS * (qs + 1)],
                            start=True, stop=True,
                            tile_position=(po, 0))
                        nc.tensor.matmul(
                            psc[64:128, QS * h:QS * (h + 1)],
                            lhsT=qk[po:po + 64, kb + 64:kb + 128],
                            rhs=qk[po:po + 64, QS * qs:QS * (qs + 1)],
                            start=True, stop=True,
                            tile_position=(po, 64))
                    if kc < 8:
                        p8, a = kc // 2, kc % 2
                        # strided write: chunk a of both heads' segments
                        dst = etA[:, 2048 * p8:2048 * (p8 + 1)].rearrange(
                            "p (h two n) -> p h two n", h=2, two=2)[:, :, a, :]
                        nc.scalar.activation(
                            dst, psc[:].rearrange("p (h n) -> p h n", h=2),
                            EXPF, scale=SCALE)
                    else:
                        j = kc - 8
                        nc.scalar.activation(
                            etB[:, 1024 * j:1024 * (j + 1)], psc[:],
                            EXPF, scale=SCALE)

            def emit_tail(prev):
                pbi, pqs, petA, petB = prev
                pv8, pv16 = v8s[pbi], v16s[pbi]
                for h in range(2):
                    pav = ps.tile([128, QS], F32, tag="ps", name="pav")
                    for p8 in range(4):
                        lhsT = pv8[:, 512 * p8 + 256 * h:
                                   512 * p8 + 256 * h + 256].rearrange(
                            "p (two m) -> p two m", two=2)
                        rhs = petA[:, 2048 * p8 + 1024 * h:
                                   2048 * p8 + 1024 * (h + 1)].rearrange(
                            "p (two n) -> p two n", two=2)
                        nc.tensor.matmul(
                            pav[:], lhsT=lhsT, rhs=rhs,
                            start=(p8 == 0), stop=False,
                            perf_mode=DR)
                    for j in range(8):
                        nc.tensor.matmul(
                            pav[:],
                            lhsT=pv16[:, 256 * j + 128 * h:
                                      256 * j + 128 * (h + 1)],
                            rhs=petB[:, 1024 * j + 512 * h:
                                     1024 * j + 512 * (h + 1)],
                            start=False, stop=(j == 7))
                    rt = spool.tile([128, QS], F32, tag="rt", name="rt")
                    # NOTE: reciprocal_approx_fast requires base partition 0
                    # (custom-DVE ucode) — run full-range; rows 0:64 are
                    # garbage reciprocals of the numerators and never read.
                    nc.vector.reciprocal_approx_fast(
                        out=rt[:], in_=pav[:])
                    ot = spool.tile([128, QS], BF16, tag="ot", name="ot")
                    nc.vector.tensor_mul(ot[0:64, :], pav[0:64, :],
                                         rt[64:128, :])
                    for half in range(2):
                        nc.sync.dma_start(
                            a2a_in[pbi][2 * pqs + half, 64 * h:64 * h + 64, :],
                            ot[0:64, QB * half:QB * (half + 1)])

            def out_pass(half):
                at = at0c if half == 0 else at_t[half]
                for qc in range(2):
                    for ns in range(2):
                        p = ps.tile([128, QS], F32, tag="ps", name="pout")
                        for cc in range(NC_):
                            nc.tensor.matmul(
                                p[:],
                                lhsT=at[:, QB * cc + 128 * qc:
                                        QB * cc + 128 * (qc + 1)],
                                rhs=w_out[:, D * cc + QS * ns:
                                          D * cc + QS * (ns + 1)],
                                start=(cc == 0), stop=(cc == NC_ - 1))
                        os_ = spool.tile([128, QS], BF16, tag="os", name="os")
                        nc.vector.scalar_tensor_tensor(
                            os_[:], p[:], zcol[:],
                            bias_b[:, QS * ns:QS * (ns + 1)],
                            ADD, ADD)
                        nc.sync.dma_start(
                            out_d[QB * half + 128 * qc:
                                  QB * half + 128 * (qc + 1),
                                  QS * ns:QS * (ns + 1)],
                            os_[:])

            # ---- emission schedule --------------------------------------
            def new_et():
                etA = epool.tile([128, 4 * 2048], FP8, tag="etA", name="etA")
                etB = epool.tile([128, 8 * 1024], BF16, tag="etB", name="etB")
                return etA, etB

            proj_block(0, 1, 0)             # b0 k0
            proj_block(0, 0, 0)             # b0 q0 (needs only x chunk 0)
            for ns in range(1, NQ):
                proj_block(0, 1, ns)        # b0 k1..k3

            # Interleave slots per attention iteration, balanced so the PE
            # work between consecutive scores_blocks stays under the Scalar
            # engine's ~18us per-iteration exp time (pre-items are emitted
            # before the deferred tail, post-items after).
            kindmap = {"q": 0, "k": 1, "v": 2}

            def do_items(items):
                for tag_, arg in items:
                    if tag_.startswith("T"):
                        bi = int(tag_[1])
                        for kc in range(*arg):
                            v_transpose_chunk(bi, kc)
                    else:
                        proj_block(int(tag_[1]), kindmap[tag_[0]], arg)

            b0_pre = [
                [],
                [("v0", 2), ("v0", 3), ("T0", (8, 16))],
                [],
                [],
            ]
            b0_post = [
                [("q0", 1), ("v0", 0), ("v0", 1), ("T0", (0, 8))],
                [("q0", 2)],
                [("q0", 3), ("k1", 0), ("k1", 1)],
                [("k1", 2), ("k1", 3), ("q1", 0)],
            ]
            prev = None
            for qs in range(NQ):
                etA, etB = new_et()
                scores_block(0, qs, etA, etB)
                do_items(b0_pre[qs])
                if prev is not None:
                    emit_tail(prev)
                prev = (0, qs, etA, etB)
                do_items(b0_post[qs])

            b1_pre = [
                [],
                [("v1", 2), ("v1", 3), ("T1", (8, 16))],
                [],
                [],
            ]
            b1_post = [
                [("v1", 0), ("v1", 1), ("T1", (0, 8)), ("q1", 1)],
                [("q1", 2)],
                [("q1", 3)],
                [],
            ]
            for qs in range(NQ):
                etA, etB = new_et()
                scores_block(1, qs, etA, etB)
                do_items(b1_pre[qs])
                emit_tail(prev)             # (b0,3) at qs==0
                if qs == 0:
                    nc.gpsimd.collective_compute(
                        "AllToAll", mybir.AluOpType.bypass,
                        replica_groups=[list(range(NC_))],
                        ins=[a2a_in[0].opt()], outs=[a2a_out[0].opt()])
                if qs == 3:
                    # Gather at0 late, AND tell the Tile scheduler these are
                    # late-timeline ops (it does not model collective
                    # latency; placed early, their PE-side sync points block
                    # the in-order PE queue mid-attention for ~25us).
                    with tc.tile_wait_until(0.16):
                        for cc in range(NC_):
                            nc.sync.dma_start(
                                at_t[0][:, QB * cc:QB * (cc + 1)],
                                a2a_out[0][cc, :, :])
                        nc.vector.tensor_copy(at0c[:], at_t[0][:])
                prev = (1, qs, etA, etB)
                do_items(b1_post[qs])

            emit_tail(prev)                 # (b1,3)
            nc.gpsimd.collective_compute(
                "AllToAll", mybir.AluOpType.bypass,
                replica_groups=[list(range(NC_))],
                ins=[a2a_in[1].opt()], outs=[a2a_out[1].opt()])
            with tc.tile_wait_until(0.20):
                out_pass(0)                 # batch-0 rows; overlaps A2A#1
            with tc.tile_wait_until(0.21):
                for cc in range(NC_):
                    nc.sync.dma_start(at_t[1][:, QB * cc:QB * (cc + 1)],
                                      a2a_out[1][cc, :, :])
            with tc.tile_wait_until(0.22):
                out_pass(1)                 # batch-1 rows

    nc.compile()
    return nc


def _chunked(a):
    """[DC*128, C] -> [128, DC*C] with chunk dc = rows 128dc:128(dc+1)."""
    r, c = a.shape
    return np.ascontiguousarray(
        a.reshape(DC, 128, c).transpose(1, 0, 2).reshape(128, DC * c))


def _shard_inputs(x, W_qkv, b_qkv, W_out, b_out):
    import ml_dtypes

    bf16 = ml_dtypes.bfloat16
    # ns-major x: xt[p, (ns*8+dc)*512 + t] = x[b][512ns+t, 128dc+p]
    xT = []
    for b in range(B):
        a = np.asarray(x[b], np.float32).astype(bf16)
        xT.append(np.ascontiguousarray(
            a.reshape(4, 512, 8, 128).transpose(3, 0, 2, 1).reshape(128, 16384)))
    W_out_bf = _chunked(W_out.astype(bf16))
    bias_bcast = np.ascontiguousarray(
        np.broadcast_to(b_out[None, :].astype(bf16), (128, D)))
    in_maps = []
    for c in range(NC_):
        lo = 64 * (2 * c)          # first channel of this core's 2 heads
        w_qk_c = _chunked(
            np.concatenate([W_qkv[:, lo:lo + 128],
                            W_qkv[:, D + lo:D + lo + 128]],
                           axis=1).astype(bf16))
        b_qv_c = np.ascontiguousarray(
            np.stack([b_qkv[lo:lo + 128],
                      b_qkv[2 * D + lo:2 * D + lo + 128]],
                     axis=1).astype(np.float32))
        w_v_c = _chunked(W_qkv[:, 2 * D + lo:2 * D + lo + 128].astype(bf16))
        in_maps.append({
            "xT0": xT[0], "xT1": xT[1],
            "w_qk": w_qk_c,
            "b_qv": b_qv_c,
            "w_v": w_v_c,
            "w_out": W_out_bf, "bias_bcast": bias_bcast,
        })
    return in_maps


def _run(inputs, trace=False, trace_kwargs=None):
    from concourse.bass_utils import run_bass_kernel_spmd

    if "nc" not in _CACHE:
        _CACHE["nc"] = _build()
    nc = _CACHE["nc"]
    in_maps = _shard_inputs(inputs["x"], inputs["W_qkv"], inputs["b_qkv"],
                            inputs["W_out"], inputs["b_out"])
    res = run_bass_kernel_spmd(nc, in_maps, core_ids=list(range(NC_)),
                               trace=trace, **(trace_kwargs or {}))
    out = np.empty((B, T, D), dtype=np.float32)
    for c in range(NC_):
        r = np.asarray(res.results[c]["out"]).astype(np.float32)
        out[0, QB * c:QB * (c + 1), :] = r[0:QB, :]
        out[1, QB * c:QB * (c + 1), :] = r[QB:2 * QB, :]
    return out, res


def kernel(x, mask, W_qkv, b_qkv, W_out, b_out):
    out, _ = _run({"x": np.asarray(x, dtype=np.float32),
                   "W_qkv": np.asarray(W_qkv, dtype=np.float32),
                   "b_qkv": np.asarray(b_qkv, dtype=np.float32),
                   "W_out": np.asarray(W_out, dtype=np.float32),
                   "b_out": np.asarray(b_out, dtype=np.float32)})
    return out


# revision 26
# speedup vs baseline: 1.0674x; 1.0321x over previous
"""Multi-head self-attention on 8 Trainium2 NeuronCores.

Problem: x[2, 2048, 1024], 16 heads x 64 dim, fp32.
  qkv = x @ W_qkv + b_qkv ; attention per head ; out = attn @ W_out + b_out

Sharding: 8-way tensor parallel over heads — core c owns heads {2c, 2c+1}
for BOTH batches.  After each batch's attention, an 8-way AllToAll on
[8, 128, 256] blocks reshards head-split -> (batch, q)-split with NO
padding: block j of batch b's A2A carries this core's two heads for
global q columns [256j, 256j+256) of batch b.  Core c ends up owning
256 output rows of EACH batch and runs the output projection as two
independent passes (pass 0 = batch-0 rows, emitted before the second
collective completes so it overlaps the A2A; pass 1 = batch-1 rows).

Schedule highlights:
  - all matmul inputs bf16 except HALF the attention AV stage (k-chunks
    0..7), which runs fp8e4 with MatmulPerfMode.DoubleRow (2 contraction
    chunks per instruction): exp writes fp8 directly from the Scalar
    engine, v is cast to fp8 on the PSUM->SBUF transpose copies.  The
    other 8 chunks stay bf16 — the mix halves the fp8 noise (rel err
    1.5e-2 vs 2.1e-2 all-fp8) while the extra PE streaming mostly hides
    under the Scalar engine's exp, which paces the attention phase.
  - x arrives host-pre-transposed in an ns-major layout and is loaded
    with 4 chunked DMAs per batch (small weight DMAs first) so the first
    projection starts after ~1MB instead of 4MB.
  - k bias dropped exactly (softmax is invariant to per-query constants)
  - scores: four quadrant-packed K=64 matmuls per k-chunk co-execute
  - softmax denominator via ones-columns interleaved into v (free PE
    columns); normalize via reciprocal_approx_fast (5x faster than the
    iterative-divide reciprocal; must run at base partition 0) + multiply
  - projections/v-transposes of the other batch are interleaved into the
    attention iterations with per-iteration budgets chosen to keep the
    emitted PE work per scores-block under the Scalar engine's ~18us
  - pass 0 of the output projection reads a DVE-copied mirror of at0
    (precise engine semaphore) so Tile's conservative DMA-queue counting
    cannot chain it behind the second collective; the at0 gather itself
    is emitted late so its A2A#0-gated DMAs never sit ahead of b1's
    attention DMAs in a queue (head-of-line stall at the b0->b1 seam)
"""

import sys
import types

# ---------------------------------------------------------------------------
# antenv.axon_hooks shim: must exist BEFORE jax initializes so the axon boot
# registers the NTFF profiling hook into it (enables trace=True timing).
if "antenv.axon_hooks" not in sys.modules:
    _m = types.ModuleType("antenv.axon_hooks")
    _m._hook = None

    def _set_hook(h, _m=_m):
        _m._hook = h

    def _get_hook(_m=_m):
        return _m._hook

    _m.set_axon_ntff_profile_hook = _set_hook
    _m.get_axon_ntff_profile_hook = _get_hook
    sys.modules["antenv.axon_hooks"] = _m
    try:
        from trn_agent_boot.trn_boot import _ntff_profile_via_ctypes

        _h = _ntff_profile_via_ctypes("/opt/axon/libaxon_pjrt.so")
        if _h is not None:
            _m._hook = _h
    except Exception:
        pass

if "/opt/trn_rl_repo" not in sys.path:
    sys.path.insert(0, "/opt/trn_rl_repo")

import numpy as np

B, T, D, H, HD = 2, 2048, 1024, 16, 64
NC_ = 8
DC = D // 128          # 8 contraction chunks for the projections
TC = T // 128          # 16 k chunks
QS = 512               # attention q-slice width
QB = 256               # a2a block q width (per destination core per batch)
NQ = T // QS           # 4 q-slices per batch
SCALE = HD ** -0.5

FP8_AV = True          # AV stage in fp8e4 + DoubleRow

_CACHE = {}


def _build(trace_enabled=False):
    import concourse.bass as bass
    import concourse.mybir as mybir
    import concourse.tile as tile
    from concourse import bacc
    from concourse.masks import make_identity

    F32 = mybir.dt.float32
    BF16 = mybir.dt.bfloat16
    FP8 = mybir.dt.float8e4
    ETD = FP8 if FP8_AV else BF16
    EXPF = mybir.ActivationFunctionType.Exp
    DR = mybir.MatmulPerfMode.DoubleRow
    ADD = mybir.AluOpType.add

    nc = bacc.Bacc("TRN2", target_bir_lowering=False, debug=False, num_devices=NC_)

    xT_d = [nc.dram_tensor(f"xT{b}", [128, 4 * DC * QS], BF16, kind="ExternalInput")
            for b in range(B)]
    w_qk_d = nc.dram_tensor("w_qk", [128, DC * 256], BF16, kind="ExternalInput")
    b_qv_d = nc.dram_tensor("b_qv", [128, 2], F32, kind="ExternalInput")
    w_v_d = nc.dram_tensor("w_v", [128, DC * 128], BF16, kind="ExternalInput")
    w_out_d = nc.dram_tensor("w_out", [128, DC * D], BF16, kind="ExternalInput")
    bias_d = nc.dram_tensor("bias_bcast", [128, D], BF16, kind="ExternalInput")
    out_d = nc.dram_tensor("out", [512, D], BF16, kind="ExternalOutput")

    with tile.TileContext(nc) as tc:
        with (
            tc.tile_pool(name="const", bufs=1) as cpool,
            tc.tile_pool(name="qk", bufs=1) as qkpool,
            tc.tile_pool(name="vt", bufs=1) as vtpool,
            tc.tile_pool(name="v", bufs=1) as vpool,
            tc.tile_pool(name="exp", bufs=2) as epool,
            tc.tile_pool(name="small", bufs=3) as spool,
            tc.tile_pool(name="at", bufs=1) as atpool,
            tc.tile_pool(name="ps", bufs=4, space="PSUM") as ps,
            tc.tile_pool(name="ps2", bufs=2, space="PSUM") as ps2,
            tc.tile_pool(name="dram", bufs=1, space="DRAM") as dram,
        ):
            # ---- constants first (small, gate the first proj), then x ----
            xts = [cpool.tile([128, 4 * DC * QS], BF16, tag=f"xt{b}",
                              name=f"xt{b}") for b in range(B)]
            w_qk = cpool.tile([128, DC * 256], BF16, tag="wqk")
            nc.sync.dma_start(w_qk[:], w_qk_d[:, :])
            w_v = cpool.tile([128, DC * 128], BF16, tag="wv")
            nc.sync.dma_start(w_v[:], w_v_d[:, :])
            # per-partition bias columns: col 0 = q bias, col 1 = v bias
            b_qv = cpool.tile([128, 2], F32, tag="bqv")
            nc.sync.dma_start(b_qv[:], b_qv_d[:, :])
            for ns in range(NQ):
                nc.sync.dma_start(xts[0][:, 4096 * ns:4096 * (ns + 1)],
                                  xT_d[0][:, 4096 * ns:4096 * (ns + 1)])
            zt = cpool.tile([128, QS], BF16, tag="zt")
            nc.vector.memset(zt[:], 0.0)
            zcol = cpool.tile([128, 1], F32, tag="zcol")
            nc.vector.memset(zcol[:], 0.0)
            ident = cpool.tile([128, 128], BF16, tag="ident")
            make_identity(nc, ident[:])
            for ns in range(NQ):
                nc.sync.dma_start(xts[1][:, 4096 * ns:4096 * (ns + 1)],
                                  xT_d[1][:, 4096 * ns:4096 * (ns + 1)])
            w_out = cpool.tile([128, DC * D], BF16, tag="wout")
            nc.sync.dma_start(w_out[:], w_out_d[:, :])
            bias_b = cpool.tile([128, D], BF16, tag="biasb")
            nc.sync.dma_start(bias_b[:], bias_d[:, :])

            a2a_in = [dram.tile([NC_, 128, QB], BF16, name=f"a2a_in{b}")
                      for b in range(B)]
            a2a_out = [dram.tile([NC_, 128, QB], BF16, name=f"a2a_out{b}")
                       for b in range(B)]

            qks = [qkpool.tile([128, 2 * T], BF16, tag=f"qk{b}", name=f"qk{b}")
                   for b in range(B)]
            vts = [vtpool.tile([128, T], BF16, tag=f"vt{b}", name=f"vt{b}")
                   for b in range(B)]
            # Mixed-precision AV: k-chunks 0..7 (pairs 0..3) run fp8e4
            # DoubleRow, chunks 8..15 run bf16 — halves the fp8 noise while
            # most of the extra PE streaming hides under the Scalar engine.
            # v8 layout per pair p8 (k chunks 2p8, 2p8+1 = A, B):
            #   [A_h0 64 | ones 64 | B_h0 | ones | A_h1 | ones | B_h1 | ones]
            # v16 layout per chunk j (kc = 8+j):
            #   [ch_h0 64 | ones 64 | ch_h1 64 | ones 64]
            v8s = [vpool.tile([128, 4 * 512], FP8, tag=f"v8{b}", name=f"v8{b}")
                   for b in range(B)]
            v16s = [vpool.tile([128, 8 * 256], BF16, tag=f"v16{b}",
                               name=f"v16{b}") for b in range(B)]
            at_t = [atpool.tile([128, NC_ * QB], BF16, tag=f"at{b}",
                                name=f"at{b}") for b in range(B)]
            # DVE-copied mirror of at0: pass-0's weight loads then wait on a
            # precise engine semaphore instead of conservative DMA-queue
            # counts (which chain behind the A2A#1-gated at1 gathers).
            at0c = atpool.tile([128, NC_ * QB], BF16, tag="at0c", name="at0c")

            def proj_block(bi, kind, ns):
                """One [128, 512] projection block: kind 0=q, 1=k, 2=v.
                q/v biases ride the PSUM->SBUF copy on the DVE; k needs no
                bias (softmax drops per-query constants)."""
                xt, qk, vt = xts[bi], qks[bi], vts[bi]
                p = ps.tile([128, QS], F32, tag="ps", name="pproj")
                for dc in range(DC):
                    if kind < 2:
                        lhsT = w_qk[:, 256 * dc + 128 * kind:
                                    256 * dc + 128 * kind + 128]
                    else:
                        lhsT = w_v[:, 128 * dc:128 * (dc + 1)]
                    nc.tensor.matmul(
                        p[:], lhsT=lhsT,
                        rhs=xt[:, 4096 * ns + 512 * dc:4096 * ns + 512 * (dc + 1)],
                        start=(dc == 0), stop=(dc == DC - 1))
                dst = vt if kind == 2 else qk
                off = QS * ns if kind == 2 else T * kind + QS * ns
                if kind == 1:
                    nc.vector.tensor_copy(dst[:, off:off + QS], p[:])
                else:
                    nc.vector.scalar_tensor_tensor(
                        dst[:, off:off + QS], p[:],
                        b_qv[:, kind // 2:kind // 2 + 1], zt[:],
                        ADD, ADD)

            def v_transpose_chunk(bi, kc):
                vt = vts[bi]
                if kc == 0:
                    nc.vector.memset(v8s[bi][:], 1.0)
                    nc.vector.memset(v16s[bi][:], 1.0)
                pt = ps.tile([128, 128], BF16, tag="ps", name="pt")
                nc.tensor.transpose(pt[:], vt[:, 128 * kc:128 * (kc + 1)],
                                    ident[:])
                if kc < 8:
                    v = v8s[bi]
                    p8, a = kc // 2, kc % 2
                    for h in range(2):
                        col = 512 * p8 + 256 * h + 128 * a
                        nc.vector.tensor_copy(v[:, col:col + 64],
                                              pt[:, 64 * h:64 * h + 64])
                else:
                    v = v16s[bi]
                    j = kc - 8
                    for h in range(2):
                        col = 256 * j + 128 * h
                        nc.vector.tensor_copy(v[:, col:col + 64],
                                              pt[:, 64 * h:64 * h + 64])

            # etA (fp8): one tile per pair p8, 2048 cols [A_h0 512|B_h0|A_h1|B_h1]
            # etB (bf16): one tile per chunk j, 1024 cols [h0 512 | h1 512]
            # Per-pair/per-chunk TILES (not one big tile): Tile tracks
            # dependencies at tile granularity, so the AV matmul for chunk k
            # can fire as soon as exp(k) lands instead of after the whole
            # q-slice's exp batch.
            def scores_block(bi, qs, etA, etB):
                qk = qks[bi]
                for kc in range(TC):
                    psc = ps2.tile([128, 2 * QS], F32, tag="ps2", name="psc")
                    kb = T + 128 * kc
                    for h in range(2):
                        po = 64 * h
                        nc.tensor.matmul(
                            psc[0:64, QS * h:QS * (h + 1)],
                            lhsT=qk[po:po + 64, kb:kb + 64],
                            rhs=qk[po:po + 64, QS * qs:QS * (qs + 1)],
                            start=True, stop=True,
                            tile_position=(po, 0))
                        nc.tensor.matmul(
                            psc[64:128, QS * h:QS * (h + 1)],
                            lhsT=qk[po:po + 64, kb + 64:kb + 128],
                            rhs=qk[po:po + 64, QS * qs:QS * (qs + 1)],
                            start=True, stop=True,
                            tile_position=(po, 64))
                    if kc < 8:
                        p8, a = kc // 2, kc % 2
                        # strided write: chunk a of both heads' segments
                        dst = etA[p8][:].rearrange(
                            "p (h two n) -> p h two n", h=2, two=2)[:, :, a, :]
                        nc.scalar.activation(
                            dst, psc[:].rearrange("p (h n) -> p h n", h=2),
                            EXPF, scale=SCALE)
                    else:
                        nc.scalar.activation(
                            etB[kc - 8][:], psc[:], EXPF, scale=SCALE)

            def emit_tail(prev):
                pbi, pqs, petA, petB = prev
                pv8, pv16 = v8s[pbi], v16s[pbi]
                for h in range(2):
                    pav = ps.tile([128, QS], F32, tag="ps", name="pav")
                    for p8 in range(4):
                        lhsT = pv8[:, 512 * p8 + 256 * h:
                                   512 * p8 + 256 * h + 256].rearrange(
                            "p (two m) -> p two m", two=2)
                        rhs = petA[p8][:, 1024 * h:1024 * (h + 1)].rearrange(
                            "p (two n) -> p two n", two=2)
                        nc.tensor.matmul(
                            pav[:], lhsT=lhsT, rhs=rhs,
                            start=(p8 == 0), stop=False,
                            perf_mode=DR)
                    for j in range(8):
                        nc.tensor.matmul(
                            pav[:],
                            lhsT=pv16[:, 256 * j + 128 * h:
                                      256 * j + 128 * (h + 1)],
                            rhs=petB[j][:, 512 * h:512 * (h + 1)],
                            start=False, stop=(j == 7))
                    rt = spool.tile([128, QS], F32, tag="rt", name="rt")
                    # NOTE: reciprocal_approx_fast requires base partition 0
                    # (custom-DVE ucode) — run full-range; rows 0:64 are
                    # garbage reciprocals of the numerators and never read.
                    nc.vector.reciprocal_approx_fast(
                        out=rt[:], in_=pav[:])
                    ot = spool.tile([128, QS], BF16, tag="ot", name="ot")
                    nc.vector.tensor_mul(ot[0:64, :], pav[0:64, :],
                                         rt[64:128, :])
                    for half in range(2):
                        nc.sync.dma_start(
                            a2a_in[pbi][2 * pqs + half, 64 * h:64 * h + 64, :],
                            ot[0:64, QB * half:QB * (half + 1)])

            def out_pass(half):
                at = at0c if half == 0 else at_t[half]
                for qc in range(2):
                    for ns in range(2):
                        p = ps.tile([128, QS], F32, tag="ps", name="pout")
                        for cc in range(NC_):
                            nc.tensor.matmul(
                                p[:],
                                lhsT=at[:, QB * cc + 128 * qc:
                                        QB * cc + 128 * (qc + 1)],
                                rhs=w_out[:, D * cc + QS * ns:
                                          D * cc + QS * (ns + 1)],
                                start=(cc == 0), stop=(cc == NC_ - 1))
                        os_ = spool.tile([128, QS], BF16, tag="os", name="os")
                        nc.vector.scalar_tensor_tensor(
                            os_[:], p[:], zcol[:],
                            bias_b[:, QS * ns:QS * (ns + 1)],
                            ADD, ADD)
                        nc.sync.dma_start(
                            out_d[QB * half + 128 * qc:
                                  QB * half + 128 * (qc + 1),
                                  QS * ns:QS * (ns + 1)],
                            os_[:])

            # ---- emission schedule --------------------------------------
            def new_et():
                etA = [epool.tile([128, 2048], FP8, tag=f"etA{p}",
                                  name=f"etA{p}") for p in range(4)]
                etB = [epool.tile([128, 1024], BF16, tag=f"etB{j}",
                                  name=f"etB{j}") for j in range(8)]
                return etA, etB

            # Head: interleave k/v projections with the 1MB x-chunk DMA
            # arrivals (each pair consumes the chunk that just landed) so
            # the PE never idles on the x load; v(b0) moving here also
            # relieves the PE-oversubscribed attention window.
            proj_block(0, 1, 0)             # b0 k0
            proj_block(0, 0, 0)             # b0 q0 (needs only x chunk 0)
            proj_block(0, 2, 0)             # b0 v0
            for ns in range(1, NQ):
                proj_block(0, 1, ns)        # b0 k_ns
                if ns < 3:
                    proj_block(0, 2, ns)    # b0 v_ns (v3 after first scores)

            # Interleave slots per attention iteration, balanced so the PE
            # work between consecutive scores_blocks stays under the Scalar
            # engine's ~18us per-iteration exp time (pre-items are emitted
            # before the deferred tail, post-items after).
            kindmap = {"q": 0, "k": 1, "v": 2}

            def do_items(items):
                for tag_, arg in items:
                    if tag_.startswith("T"):
                        bi = int(tag_[1])
                        for kc in range(*arg):
                            v_transpose_chunk(bi, kc)
                    else:
                        proj_block(int(tag_[1]), kindmap[tag_[0]], arg)

            b0_pre = [
                [],
                [("T0", (8, 16))],
                [],
                [],
            ]
            b0_post = [
                [("v0", 3), ("q0", 1), ("T0", (0, 8))],
                [("q0", 2)],
                [("q0", 3), ("k1", 0), ("k1", 1)],
                [("k1", 2), ("k1", 3), ("q1", 0)],
            ]
            prev = None
            for qs in range(NQ):
                etA, etB = new_et()
                scores_block(0, qs, etA, etB)
                do_items(b0_pre[qs])
                if prev is not None:
                    emit_tail(prev)
                prev = (0, qs, etA, etB)
                do_items(b0_post[qs])

            b1_pre = [
                [],
                [("v1", 2), ("v1", 3), ("T1", (8, 16))],
                [],
                [],
            ]
            b1_post = [
                [("v1", 0), ("v1", 1), ("T1", (0, 8)), ("q1", 1)],
                [("q1", 2)],
                [("q1", 3)],
                [],
            ]
            for qs in range(NQ):
                etA, etB = new_et()
                scores_block(1, qs, etA, etB)
                do_items(b1_pre[qs])
                emit_tail(prev)             # (b0,3) at qs==0
                if qs == 0:
                    nc.gpsimd.collective_compute(
                        "AllToAll", mybir.AluOpType.bypass,
                        replica_groups=[list(range(NC_))],
                        ins=[a2a_in[0].opt()], outs=[a2a_out[0].opt()])
                if qs == 3:
                    # Gather at0 late, AND tell the Tile scheduler these are
                    # late-timeline ops (it does not model collective
                    # latency; placed early, their PE-side sync points block
                    # the in-order PE queue mid-attention for ~25us).
                    with tc.tile_wait_until(0.16):
                        for cc in range(NC_):
                            nc.sync.dma_start(
                                at_t[0][:, QB * cc:QB * (cc + 1)],
                                a2a_out[0][cc, :, :])
                        nc.vector.tensor_copy(at0c[:], at_t[0][:])
                prev = (1, qs, etA, etB)
                do_items(b1_post[qs])

            emit_tail(prev)                 # (b1,3)
            nc.gpsimd.collective_compute(
                "AllToAll", mybir.AluOpType.bypass,
                replica_groups=[list(range(NC_))],
                ins=[a2a_in[1].opt()], outs=[a2a_out[1].opt()])
            with tc.tile_wait_until(0.20):
                out_pass(0)                 # batch-0 rows; overlaps A2A#1
            with tc.tile_wait_until(0.21):
                for cc in range(NC_):
                    nc.sync.dma_start(at_t[1][:, QB * cc:QB * (cc + 1)],
                                      a2a_out[1][cc, :, :])
            with tc.tile_wait_until(0.22):
                out_pass(1)                 # batch-1 rows

    nc.compile()
    return nc


def _chunked(a):
    """[DC*128, C] -> [128, DC*C] with chunk dc = rows 128dc:128(dc+1)."""
    r, c = a.shape
    return np.ascontiguousarray(
        a.reshape(DC, 128, c).transpose(1, 0, 2).reshape(128, DC * c))


def _shard_inputs(x, W_qkv, b_qkv, W_out, b_out):
    import ml_dtypes

    bf16 = ml_dtypes.bfloat16
    # ns-major x: xt[p, (ns*8+dc)*512 + t] = x[b][512ns+t, 128dc+p]
    xT = []
    for b in range(B):
        a = np.asarray(x[b], np.float32).astype(bf16)
        xT.append(np.ascontiguousarray(
            a.reshape(4, 512, 8, 128).transpose(3, 0, 2, 1).reshape(128, 16384)))
    W_out_bf = _chunked(W_out.astype(bf16))
    bias_bcast = np.ascontiguousarray(
        np.broadcast_to(b_out[None, :].astype(bf16), (128, D)))
    in_maps = []
    for c in range(NC_):
        lo = 64 * (2 * c)          # first channel of this core's 2 heads
        w_qk_c = _chunked(
            np.concatenate([W_qkv[:, lo:lo + 128],
                            W_qkv[:, D + lo:D + lo + 128]],
                           axis=1).astype(bf16))
        b_qv_c = np.ascontiguousarray(
            np.stack([b_qkv[lo:lo + 128],
                      b_qkv[2 * D + lo:2 * D + lo + 128]],
                     axis=1).astype(np.float32))
        w_v_c = _chunked(W_qkv[:, 2 * D + lo:2 * D + lo + 128].astype(bf16))
        in_maps.append({
            "xT0": xT[0], "xT1": xT[1],
            "w_qk": w_qk_c,
            "b_qv": b_qv_c,
            "w_v": w_v_c,
            "w_out": W_out_bf, "bias_bcast": bias_bcast,
        })
    return in_maps


def _run(inputs, trace=False, trace_kwargs=None):
    from concourse.bass_utils import run_bass_kernel_spmd

    if "nc" not in _CACHE:
        _CACHE["nc"] = _build()
    nc = _CACHE["nc"]
    in_maps = _shard_inputs(inputs["x"], inputs["W_qkv"], inputs["b_qkv"],
                            inputs["W_out"], inputs["b_out"])
    res = run_bass_kernel_spmd(nc, in_maps, core_ids=list(range(NC_)),
                               trace=trace, **(trace_kwargs or {}))
    out = np.empty((B, T, D), dtype=np.float32)
    for c in range(NC_):
        r = np.asarray(res.results[c]["out"]).astype(np.float32)
        out[0, QB * c:QB * (c + 1), :] = r[0:QB, :]
        out[1, QB * c:QB * (c + 1), :] = r[QB:2 * QB, :]
    return out, res


def kernel(x, mask, W_qkv, b_qkv, W_out, b_out):
    out, _ = _run({"x": np.asarray(x, dtype=np.float32),
                   "W_qkv": np.asarray(W_qkv, dtype=np.float32),
                   "b_qkv": np.asarray(b_qkv, dtype=np.float32),
                   "W_out": np.asarray(W_out, dtype=np.float32),
                   "b_out": np.asarray(b_out, dtype=np.float32)})
    return out
